# revision 1
# baseline (speedup 1.0000x reference)
"""Trainium2 Bass kernel for nn_ContentLoss (Gaussian-blur content MSE).

Math: reference computes MSE( blur61(a).mean(ch), blur61(b).mean(ch) ) with a
61x61 Gaussian (sigma=1, separable) and reflect padding.  Everything before
the final square is linear, so each core computes

    g = colconv(rowconv(mean_ch(a - b)))         (per image)

as two banded-matrix multiplies on the TensorEngine:
  pass 1 folds the channel combine (+-1/3) into PSUM accumulation and emits
  the column-convolved image *transposed* (y1T = x.T @ B.T per plane chunk),
  pass 2 applies the same banded matrix to the transposed intermediate.
The 61-tap kernel with sigma=1 has fp32-significant support of ~+-13 taps, so
chunk-to-chunk coupling is fully covered by 16-col windows (pass 1) and 32-row
slivers (pass 2); the result is exact up to fp32 rounding.

Sharding: pure data parallel, 2 images per core across 8 cores.  Each core
returns per-partition partial sums of g**2; the host reduces in float64.
"""

import sys

import numpy as np

if "/opt/trn_rl_repo" not in sys.path:
    sys.path.insert(0, "/opt/trn_rl_repo")

N = 512
P = 128
S = 16  # pass-1 h_out window halo
IMGS_PER_CORE = 2
N_CORES = 8
FULL_BATCH = 16


def _build_B():
    """B[i, j]: 1D 61-tap normalized Gaussian conv matrix with reflect pad."""
    x = np.arange(61, dtype=np.float64)
    k1 = np.exp(-((x - 30.0) ** 2) / 2.0)
    k1n = k1 / k1.sum()
    B = np.zeros((N, N), np.float64)
    i = np.arange(N)
    for t in range(61):
        j = i + (t - 30)
        j = np.abs(j)
        j = np.where(j > N - 1, 2 * (N - 1) - j, j)
        np.add.at(B, (i, j), k1n[t])
    return B


def _windows():
    return [
        (max(0, 128 * c - S), min(N, 128 * c + 128 + S)) for c in range(4)
    ]


def _build_consts():
    BT = _build_B().T.copy()
    W = _windows()
    wmax = max(hi - lo for lo, hi in W)
    c1p = np.zeros((P, 4, wmax), np.float32)
    for c, (lo, hi) in enumerate(W):
        c1p[:, c, : hi - lo] = (BT[128 * c : 128 * (c + 1), lo:hi] / 3.0)
    c1m = -c1p
    c2main = np.zeros((P, 4, 128), np.float32)
    for m in range(4):
        c2main[:, m, :] = BT[128 * m : 128 * (m + 1), 128 * m : 128 * (m + 1)]
    # full off-diagonal blocks: BT[chunk m-1, chunk m] and BT[chunk m+1, chunk m]
    # (only ~30 rows near the boundary are nonzero; K=128 costs the same as
    # K=32 on the PE, and full blocks avoid partition-offset operands)
    c2prev = np.zeros((P, 3, 128), np.float32)
    for m in range(1, 4):
        c2prev[:, m - 1, :] = BT[
            128 * (m - 1) : 128 * m, 128 * m : 128 * (m + 1)
        ]
    c2next = np.zeros((P, 3, 128), np.float32)
    for m in range(0, 3):
        c2next[:, m, :] = BT[
            128 * (m + 1) : 128 * (m + 2), 128 * m : 128 * (m + 1)
        ]
    return c1p, c1m, c2main, c2prev, c2next


def build_nc(const_inline=True, slivers=True):
    from contextlib import ExitStack

    import concourse.bacc as bacc
    import concourse.tile as tile
    from concourse import mybir

    f32 = mybir.dt.float32
    nc = bacc.Bacc(
        "TRN2", target_bir_lowering=False, debug=False, num_devices=N_CORES
    )

    a = nc.dram_tensor("a", [IMGS_PER_CORE, 3, N, N], f32, kind="ExternalInput")
    b = nc.dram_tensor("b", [IMGS_PER_CORE, 3, N, N], f32, kind="ExternalInput")
    # out[p, 4*img+m] = partial sum over (h) of gT[m*128+p-chunk, :]**2
    out = nc.dram_tensor(
        "out", [P, 4 * IMGS_PER_CORE], f32, kind="ExternalOutput"
    )

    c1p_np, c1m_np, c2main_np, c2prev_np, c2next_np = _build_consts()
    if const_inline:
        c1p_d = nc.inline_tensor(c1p_np, name="c1p")
        c1m_d = nc.inline_tensor(c1m_np, name="c1m")
        c2main_d = nc.inline_tensor(c2main_np, name="c2main")
        c2prev_d = nc.inline_tensor(c2prev_np, name="c2prev")
        c2next_d = nc.inline_tensor(c2next_np, name="c2next")
    else:
        c1p_d = nc.dram_tensor("c1p", list(c1p_np.shape), f32, kind="ExternalInput")
        c1m_d = nc.dram_tensor("c1m", list(c1m_np.shape), f32, kind="ExternalInput")
        c2main_d = nc.dram_tensor(
            "c2main", list(c2main_np.shape), f32, kind="ExternalInput"
        )
        c2prev_d = nc.dram_tensor(
            "c2prev", list(c2prev_np.shape), f32, kind="ExternalInput"
        )
        c2next_d = nc.dram_tensor(
            "c2next", list(c2next_np.shape), f32, kind="ExternalInput"
        )

    W = _windows()
    wmax = c1p_np.shape[2]

    with tile.TileContext(nc) as tc, ExitStack() as ctx:
        consts = ctx.enter_context(tc.tile_pool(name="consts", bufs=1))
        planes = ctx.enter_context(tc.tile_pool(name="planes", bufs=12))
        y1pool = ctx.enter_context(tc.tile_pool(name="y1pool", bufs=8))
        accp = ctx.enter_context(tc.tile_pool(name="accp", bufs=1))
        scratchp = ctx.enter_context(tc.tile_pool(name="scratchp", bufs=2))
        psum1 = ctx.enter_context(tc.tile_pool(name="psum1", bufs=4, space="PSUM"))
        psum2 = ctx.enter_context(tc.tile_pool(name="psum2", bufs=4, space="PSUM"))

        c1p_t = consts.tile([P, 4, wmax], f32, name="c1p_t")
        nc.sync.dma_start(out=c1p_t, in_=c1p_d.ap())
        c1m_t = consts.tile([P, 4, wmax], f32, name="c1m_t")
        nc.sync.dma_start(out=c1m_t, in_=c1m_d.ap())
        c2main_t = consts.tile([P, 4, 128], f32, name="c2main_t")
        nc.sync.dma_start(out=c2main_t, in_=c2main_d.ap())
        c2prev_t = consts.tile([P, 3, 128], f32, name="c2prev_t")
        nc.sync.dma_start(out=c2prev_t, in_=c2prev_d.ap())
        c2next_t = consts.tile([P, 3, 128], f32, name="c2next_t")
        nc.sync.dma_start(out=c2next_t, in_=c2next_d.ap())

        acc_t = accp.tile([P, 4 * IMGS_PER_CORE], f32, name="acc_t")

        for img in range(IMGS_PER_CORE):
            # --- load the 6 planes of this image (a ch0..2, then b ch0..2)
            plane_ts = []
            for src, src_name in ((a, "a"), (b, "b")):
                for ch in range(3):
                    pt = planes.tile(
                        [P, 4, N], f32, name=f"pl_{src_name}{img}c{ch}", tag="pl"
                    )
                    nc.sync.dma_start(
                        out=pt,
                        in_=src.ap()[img, ch].rearrange("(c p) w -> p c w", p=P),
                    )
                    plane_ts.append(pt)

            # --- pass 1: y1T[w, h_out] per w-chunk, channel-combine in PSUM
            ps1 = [
                psum1.tile([P, N], f32, name=f"ps1_{img}_{wc}", tag="ps1")
                for wc in range(4)
            ]
            # plane 0 writes the bank in disjoint segments (start=True marks the
            # whole 2KB zero-region pending; each matmul must touch uniformly
            # pending or uniformly written bytes), later planes accumulate.
            for pi in range(6):
                coef_t = c1p_t if pi < 3 else c1m_t
                for wc in range(4):
                    for c in range(4):
                        lo, hi = W[c]
                        if pi == 0:
                            # fresh segment ([lo,hi) minus the 32-wide strip
                            # already written by chunk c-1), then the overlap
                            # strip accumulated separately
                            fresh_lo = lo if c == 0 else 128 * c + S
                            segs = [(fresh_lo, hi, c == 0)]
                            if c > 0:
                                segs.append((128 * c - S, 128 * c + S, False))
                        else:
                            segs = [(lo, hi, False)]
                        for seg_lo, seg_hi, is_start in segs:
                            nc.tensor.matmul(
                                ps1[wc][:, seg_lo:seg_hi],
                                lhsT=plane_ts[pi][:, c, 128 * wc : 128 * (wc + 1)],
                                rhs=coef_t[:, c, seg_lo - lo : seg_hi - lo],
                                start=is_start,
                                stop=(pi == 5 and c == 3),
                            )

            y1 = []
            for wc in range(4):
                yt = y1pool.tile([P, N], f32, name=f"y1_{img}_{wc}", tag="y1")
                nc.scalar.copy(yt, ps1[wc])
                y1.append(yt)

            # --- pass 2: gT chunk per w_out chunk m, then square+row-reduce
            for m in range(4):
                ps2 = psum2.tile([P, N], f32, name=f"ps2_{img}_{m}", tag="ps2")
                n_mm = 1 + (slivers and m > 0) + (slivers and m < 3)
                k = 0
                nc.tensor.matmul(
                    ps2,
                    lhsT=c2main_t[:, m, :],
                    rhs=y1[m],
                    start=True,
                    stop=(k := k + 1) == n_mm,
                )
                if slivers and m > 0:
                    nc.tensor.matmul(
                        ps2,
                        lhsT=c2prev_t[:, m - 1, :],
                        rhs=y1[m - 1],
                        start=False,
                        stop=(k := k + 1) == n_mm,
                    )
                if slivers and m < 3:
                    nc.tensor.matmul(
                        ps2,
                        lhsT=c2next_t[:, m, :],
                        rhs=y1[m + 1],
                        start=False,
                        stop=(k := k + 1) == n_mm,
                    )
                scr = scratchp.tile([P, N], f32, name=f"scr_{img}_{m}", tag="scr")
                nc.scalar.activation(
                    scr,
                    ps2,
                    mybir.ActivationFunctionType.Square,
                    accum_out=acc_t[:, 4 * img + m : 4 * img + m + 1],
                )

        nc.sync.dma_start(out=out.ap(), in_=acc_t)

    nc.finalize()
    return nc


_CACHE = {}


def _get_nc(**opts):
    key = tuple(sorted(opts.items()))
    if key not in _CACHE:
        _CACHE[key] = build_nc(**opts)
    return _CACHE[key]


def run(inputs, const_inline=True, slivers=True, **spmd_kwargs):
    """Run on 8 cores; returns (scalar_result, BassKernelResults)."""
    from concourse import bass_utils

    a = np.ascontiguousarray(np.asarray(inputs["a"], dtype=np.float32))
    b = np.ascontiguousarray(np.asarray(inputs["b"], dtype=np.float32))
    assert a.shape == (FULL_BATCH, 3, N, N) and b.shape == a.shape

    nc = _get_nc(const_inline=const_inline, slivers=slivers)
    const_map = {}
    if not const_inline:
        names = ["c1p", "c1m", "c2main", "c2prev", "c2next"]
        const_map = dict(zip(names, _build_consts()))
    in_maps = []
    for core in range(N_CORES):
        sl = slice(core * IMGS_PER_CORE, (core + 1) * IMGS_PER_CORE)
        in_maps.append(
            {
                "a": np.ascontiguousarray(a[sl]),
                "b": np.ascontiguousarray(b[sl]),
                **const_map,
            }
        )
    res = bass_utils.run_bass_kernel_spmd(
        nc, in_maps, core_ids=list(range(N_CORES)), **spmd_kwargs
    )
    total = 0.0
    for r in res.results:
        total += np.asarray(r["out"]).astype(np.float64).sum()
    mse = np.float32(total / (FULL_BATCH * N * N))
    return np.asarray(mse, dtype=np.float32), res


def kernel(**inputs) -> np.ndarray:
    result, _ = run(inputs)
    return result

